# revision 10
# baseline (speedup 1.0000x reference)
"""Bahdanau-attention kernel for Trainium2 (8 NeuronCores, SPMD data-parallel).

Computes softmax(mask(v . tanh(enc @ W_h^T + dec @ W_s^T + b_h + b_s))) for
B=64, S=4096, H=512, E=1024.  Sharded data-parallel over batch: 8 batches per
core, weights replicated.  All heavy compute (the [32768,1024]@[1024,512]
projection per core) runs on the TensorEngine in bf16; softmax runs on
ScalarE/VectorE without a max-subtraction (|scores| <= sum|v| ~ 11.3, so exp
cannot overflow).
"""

import sys

import numpy as np

try:
    import concourse.bass as bass  # noqa: F401
except ImportError:  # pragma: no cover
    sys.path.insert(0, "/opt/trn_rl_repo")
    import concourse.bass as bass  # noqa: F401

import ml_dtypes

BF16 = ml_dtypes.bfloat16

B, S, H = 64, 4096, 512
E = 2 * H  # 1024
NCORES = 8
NB = B // NCORES  # 8 local batches per core
NSC = S // 512  # 8 sequence chunks of 512 per batch
NKC = E // 128  # 8 contraction chunks
NHC = H // 128  # 4 h chunks
NEC = 4  # contraction chunks for the dec projection (512/128)

TRACE = False
LAST_EXEC_NS = None
LAST_RESULTS = None
_CACHE = {}


def const_layout(n_b=NB, n_sc=NSC):
    """Column offsets inside the single bf16 constants tensor [128, CW]."""
    off = {}
    off["wt"] = 0  # [p, kc*H + h] = W_h[h, kc*128+p]
    off["vemb"] = off["wt"] + NKC * H  # [p, (hc*n_sc+sc)*n_sc + m]
    off["wst"] = off["vemb"] + NHC * n_sc * n_sc  # [p, ec*H + hc*128 + h]
    off["dect"] = off["wst"] + NEC * H  # [p, ec*n_b + b]   (per-core)
    off["whb"] = off["dect"] + NEC * n_b  # row 0 only: [0, hc*128+h]
    off["ones"] = off["whb"] + H  # row 0 only: [0, 0:n_b] = 1.0
    off["_width"] = off["ones"] + n_b
    return off


def build_bass(n_b=NB, n_sc=NSC):
    import concourse.bass as bass
    import concourse.tile as tile
    from concourse import mybir
    from contextlib import ExitStack

    f32 = mybir.dt.float32
    bf16 = mybir.dt.bfloat16
    Tanh = mybir.ActivationFunctionType.Tanh
    Exp = mybir.ActivationFunctionType.Exp
    mult = mybir.AluOpType.mult
    add = mybir.AluOpType.add

    ncols = n_b * n_sc * 512
    lay = const_layout(n_b, n_sc)
    CW = lay["_width"]
    FW = n_b * 512 + n_sc  # f32 tensor: mask then ones columns

    nc = bass.Bass()
    enc_ext = nc.declare_dram_parameter("encT", [128, NKC, ncols], bf16, False)
    cst_ext = nc.declare_dram_parameter("consts", [128, CW], bf16, False)
    f32_ext = nc.declare_dram_parameter("f32c", [n_sc, FW], f32, False)
    out_ext = nc.declare_dram_parameter("out", [n_b, n_sc, 512], f32, True)

    with ExitStack() as ctx:
        tc = ctx.enter_context(tile.TileContext(nc))
        const = ctx.enter_context(tc.tile_pool(name="const", bufs=1))
        encp = ctx.enter_context(tc.tile_pool(name="enc", bufs=4))
        xp = ctx.enter_context(tc.tile_pool(name="x", bufs=8))
        bp = ctx.enter_context(tc.tile_pool(name="bt", bufs=8))
        psmm = ctx.enter_context(tc.tile_pool(name="psmm", bufs=3, space="PSUM"))
        pssc = ctx.enter_context(tc.tile_pool(name="pssc", bufs=2, space="PSUM"))
        psb = ctx.enter_context(tc.tile_pool(name="psb", bufs=2, space="PSUM"))
        psdec_p = ctx.enter_context(tc.tile_pool(name="psdec", bufs=1, space="PSUM"))

        cst = const.tile([128, CW], bf16, tag="cst")
        nc.sync.dma_start(cst[:], cst_ext[:])
        f32_0 = const.tile([n_sc, FW], f32, tag="f32_0")
        nc.sync.dma_start(f32_0[:], f32_ext[:])
        # re-home the f32 constants onto ScalarE: the VectorE reduce then waits
        # on the ScalarE semaphore alone (which also covers exp outputs)
        f32c = const.tile([n_sc, FW], f32, tag="f32c")
        nc.scalar.copy(f32c[:], f32_0[:])

        def cs(name, lo, width):
            return cst[:, lay[name] + lo : lay[name] + lo + width]

        # Preload the exp_and_others ACT table set (tanh+exp share it) so the
        # implicit table-load pseudo doesn't ride on a hot-loop instruction.
        warm = const.tile([1, 3], f32, tag="warm")
        nc.scalar.activation(warm[:, 0:1], f32_0[0:1, 0:1], Tanh)
        nc.scalar.activation(warm[:, 1:2], f32_0[0:1, 0:1], Exp)

        # dec projection: bias[h, b] = sum_e W_s[h,e] dec[b,e] + (b_h+b_s)[h]
        psdec = psdec_p.tile([128, NHC * n_b], f32, tag="psdec")
        for hc in range(NHC):
            for ec in range(NEC):
                nc.tensor.matmul(
                    psdec[:, hc * n_b : (hc + 1) * n_b],
                    cs("wst", ec * H + hc * 128, 128),
                    cs("dect", ec * n_b, n_b),
                    start=(ec == 0),
                    stop=False,
                )
            nc.tensor.matmul(
                psdec[:, hc * n_b : (hc + 1) * n_b],
                cst[0:1, lay["whb"] + hc * 128 : lay["whb"] + (hc + 1) * 128],
                cst[0:1, lay["ones"] : lay["ones"] + n_b],
                start=False,
                stop=True,
            )
        # evacuate on VectorE, then touch once on ScalarE: the hot-loop tanh
        # then needs only its PE wait (hw instructions hold a single wait)
        bias_sb = const.tile([128, NHC * n_b], f32, tag="bias")
        nc.vector.tensor_copy(bias_sb[:], psdec[:])
        nc.scalar.copy(warm[:, 2:3], bias_sb[0:1, 0:1])

        for b in range(n_b):
            ps_sc = pssc.tile([n_sc, 512], f32, tag="ps_sc")
            for sc in range(n_sc):
                c0 = (b * n_sc + sc) * 512
                et = encp.tile([128, NKC, 512], bf16, tag="et")
                nc.sync.dma_start(et[:], enc_ext[:, :, c0 : c0 + 512])
                for hc in range(NHC):
                    ps = psmm.tile([128, 512], f32, tag="ps")
                    for kc in range(NKC):
                        nc.tensor.matmul(
                            ps[:],
                            cs("wt", kc * H + hc * 128, 128),
                            et[:, kc, :],
                            start=(kc == 0),
                            stop=(kc == NKC - 1),
                        )
                    xt = xp.tile([128, 512], bf16, tag="xt")
                    nc.scalar.activation(
                        xt[:],
                        ps[:],
                        Tanh,
                        bias=bias_sb[:, hc * n_b + b : hc * n_b + b + 1],
                    )
                    nc.tensor.matmul(
                        ps_sc[:],
                        cs("vemb", (hc * n_sc + sc) * n_sc, n_sc),
                        xt[:],
                        start=(sc == 0 and hc == 0),
                        stop=(sc == n_sc - 1 and hc == NHC - 1),
                    )
                # ScalarE observes its own newest tick so recycled xt slots
                # never add a second (same-engine WAW) wait to a later tanh
                nc.scalar.copy(warm[:, 2:3], xt[0:1, 0:1])
            # batch epilogue: masked softmax over the n_sc*512 scores of batch b
            u8 = bp.tile([n_sc, 512], f32, tag="u8")
            nc.scalar.activation(u8[:], ps_sc[:], Exp)
            uw = bp.tile([n_sc, 512], f32, tag="uw")
            part = bp.tile([n_sc, 1], f32, tag="part")
            nc.vector.tensor_mul(uw[:], u8[:], f32c[:, b * 512 : (b + 1) * 512])
            nc.vector.tensor_reduce(
                part[:], uw[:], axis=mybir.AxisListType.X, op=add
            )
            tot_ps = psb.tile([n_sc, 1], f32, tag="tot")
            nc.tensor.matmul(tot_ps[:], f32c[:, n_b * 512 : n_b * 512 + n_sc], part[:])
            recip = bp.tile([n_sc, 1], f32, tag="recip")
            nc.vector.reciprocal(recip[:], tot_ps[:])
            probs = bp.tile([n_sc, 512], f32, tag="probs")
            nc.vector.tensor_scalar_mul(probs[:], uw[:], recip[:])
            nc.sync.dma_start(out_ext[b], probs[:])

    return nc


def legalize_single_wait(nc):
    """The walrus in this container accepts at most ONE sync wait per
    instruction and cannot encode EVENT_SEMAPHORE_RANGE_CLEAR.  Split excess
    waits onto single-wait NOPs, and replace the tile-exit range clear with
    per-semaphore decrements of the statically known final values."""
    import concourse.mybir as mybir
    import bass_rust

    m = nc.m
    totals = {}
    names = {}
    for fn in m.functions:
        for blk in fn.blocks:
            for inst in blk.instructions:
                si = getattr(inst, "sync_info", None)
                if not si:
                    continue
                for u in si.on_update or []:
                    if u.sync_type != "semaphore":
                        continue
                    v = u.update_value if u.update_value is not None else 1
                    if u.update_mode in ("sem-inc", "sem-add-imm"):
                        totals[u.id] = totals.get(u.id, 0) + v
                    elif u.update_mode in ("sem-dec", "sem-sub-imm"):
                        totals[u.id] = totals.get(u.id, 0) - v
                    names[u.id] = u.ant_name

    nid = [0]

    def mk_nop(engine, wait):
        nid[0] += 1
        nop = mybir.InstNoOp(name=f"I-lsw-{nid[0]}", engine=engine, ins=[], outs=[])
        nop.sync_info = bass_rust.SyncInfo(on_wait=[wait], on_update=[])
        return nop

    def mk_dec(engine, sem_id, value):
        nid[0] += 1
        es = mybir.InstEventSemaphore(
            name=f"I-lsc-{nid[0]}", engine=engine, ins=[], outs=[]
        )
        u = bass_rust.SyncUpdate(
            sync_type="semaphore",
            id=sem_id,
            ant_name=names.get(sem_id, f"sem{sem_id}"),
            update_mode="sem-sub-imm",
            update_value=value,
            update_reg=None,
        )
        es.sync_info = bass_rust.SyncInfo(on_wait=[], on_update=[u])
        return es

    for fn in m.functions:
        for blk in fn.blocks:
            out = []
            for inst in blk.instructions:
                if (
                    isinstance(inst, mybir.InstISA)
                    and getattr(inst, "isa_opcode", None) == 176
                ):
                    first = getattr(inst, "range_first", None)
                    last = getattr(inst, "range_last", None)
                    if first is None:
                        d = inst.concise()
                        import re

                        first = int(re.search(r"range_first=(\d+)", d).group(1))
                        last = int(re.search(r"range_last=(\d+)", d).group(1))
                    for sem_id in range(first, last + 1):
                        v = totals.get(sem_id, 0)
                        if v > 0:
                            out.append(mk_dec(inst.engine, sem_id, v))
                    continue
                si = getattr(inst, "sync_info", None)
                waits = list(si.on_wait) if si and si.on_wait else []
                if len(waits) > 1:
                    for w in waits[:-1]:
                        out.append(mk_nop(inst.engine, w))
                    inst.sync_info = bass_rust.SyncInfo(
                        on_wait=[waits[-1]], on_update=list(si.on_update or [])
                    )
                out.append(inst)
            blk.instructions = out


def prep_shared(W_h_w, W_h_b, W_s_w, W_s_b, v_w, n_b=NB, n_sc=NSC):
    """The shared part of the constants tensor (zero where dect goes)."""
    lay = const_layout(n_b, n_sc)
    Wh = np.asarray(W_h_w, np.float32)  # [H, E]
    Ws = np.asarray(W_s_w, np.float32)  # [H, H]
    v = np.asarray(v_w, np.float32).reshape(H)

    cst = np.zeros((128, lay["_width"]), np.float32)
    # wt[p, kc*H + h] = Wh[h, kc*128+p]
    cst[:, lay["wt"] : lay["wt"] + NKC * H] = (
        Wh.T.reshape(NKC, 128, H).transpose(1, 0, 2).reshape(128, NKC * H)
    )
    # vemb[p, (hc*n_sc+sc)*n_sc + m] = (m==sc) * v[hc*128+p]
    vr = v.reshape(NHC, 128).T  # [p, hc]
    vemb = np.zeros((128, NHC, n_sc, n_sc), np.float32)
    for scm in range(n_sc):
        vemb[:, :, scm, scm] = vr
    cst[:, lay["vemb"] : lay["vemb"] + NHC * n_sc * n_sc] = vemb.reshape(128, -1)
    # wst[p, ec*H + hc*128 + h] = Ws[hc*128+h, ec*128+p]
    cst[:, lay["wst"] : lay["wst"] + NEC * H] = (
        Ws.T.reshape(NEC, 128, NHC, 128).transpose(1, 0, 2, 3).reshape(128, NEC * H)
    )
    # whb row 0: combined bias, ones row 0
    cst[0, lay["whb"] : lay["whb"] + H] = np.asarray(W_h_b, np.float32) + np.asarray(
        W_s_b, np.float32
    )
    cst[0, lay["ones"] : lay["ones"] + n_b] = 1.0
    return cst


def prep_core(cst_shared, dec_shard, enc_shard, mask_shard, n_b=NB, n_sc=NSC):
    """Per-core input map (layout transforms + bf16 cast only)."""
    lay = const_layout(n_b, n_sc)
    ncols = n_b * n_sc * 512
    rows = np.asarray(enc_shard, np.float32).reshape(ncols, E).astype(BF16)
    # encT[p, kc, col] = enc[col, kc*128+p]
    encT = np.ascontiguousarray(rows.T.reshape(NKC, 128, ncols).transpose(1, 0, 2))
    cst = cst_shared.copy()
    # dect[p, ec*n_b + b] = dec[b, ec*128+p]
    cst[:, lay["dect"] : lay["dect"] + NEC * n_b] = (
        np.asarray(dec_shard, np.float32)
        .T.reshape(NEC, 128, n_b)
        .transpose(1, 0, 2)
        .reshape(128, NEC * n_b)
    )
    # f32c: mask columns then ones columns
    f32c = np.ones((n_sc, n_b * 512 + n_sc), np.float32)
    f32c[:, : n_b * 512] = (
        np.asarray(mask_shard)
        .astype(np.float32)
        .reshape(n_b, n_sc, 512)
        .transpose(1, 0, 2)
        .reshape(n_sc, n_b * 512)
    )
    return {"encT": encT, "consts": cst.astype(BF16), "f32c": f32c}


def kernel(decoder_output, encoder_output, mask, W_h_w, W_h_b, W_s_w, W_s_b, v_w):
    global LAST_EXEC_NS, LAST_RESULTS
    from concourse.bass_utils import run_bass_kernel_spmd

    if "nc" not in _CACHE:
        nc0 = build_bass()
        nc0.finalize()
        legalize_single_wait(nc0)
        _CACHE["nc"] = nc0
    nc = _CACHE["nc"]

    shared = prep_shared(W_h_w, W_h_b, W_s_w, W_s_b, v_w)
    in_maps = []
    for c in range(NCORES):
        sl = slice(c * NB, (c + 1) * NB)
        in_maps.append(
            prep_core(shared, decoder_output[sl], encoder_output[sl], mask[sl])
        )

    res = run_bass_kernel_spmd(nc, in_maps, core_ids=list(range(NCORES)), trace=TRACE)
    if TRACE:
        LAST_EXEC_NS = res.exec_time_ns
        LAST_RESULTS = res
    out = np.concatenate(
        [
            np.asarray(res.results[c]["out"], np.float32).reshape(NB, S)
            for c in range(NCORES)
        ],
        axis=0,
    )
    return out


# revision 11
# speedup vs baseline: 1.5750x; 1.5750x over previous
"""Bahdanau-attention kernel for Trainium2 (8 NeuronCores, SPMD data-parallel).

Computes softmax(mask(v . tanh(enc @ W_h^T + dec @ W_s^T + b_h + b_s))) for
B=64, S=4096, H=512, E=1024.  Sharded data-parallel over batch: 8 batches per
core, weights replicated.  All heavy compute (the [32768,1024]@[1024,512]
projection per core) runs on the TensorEngine in bf16; softmax runs on
ScalarE/VectorE without a max-subtraction (|scores| <= sum|v| ~ 11.3, so exp
cannot overflow).
"""

import sys

import numpy as np

try:
    import concourse.bass as bass  # noqa: F401
except ImportError:  # pragma: no cover
    sys.path.insert(0, "/opt/trn_rl_repo")
    import concourse.bass as bass  # noqa: F401

import ml_dtypes

BF16 = ml_dtypes.bfloat16
FP8 = ml_dtypes.float8_e4m3
USE_FP8 = True

B, S, H = 64, 4096, 512
E = 2 * H  # 1024
NCORES = 8
NB = B // NCORES  # 8 local batches per core
NSC = S // 512  # 8 sequence chunks of 512 per batch
NKC = E // 128  # 8 contraction chunks
NHC = H // 128  # 4 h chunks
NEC = 4  # contraction chunks for the dec projection (512/128)

TRACE = False
LAST_EXEC_NS = None
LAST_RESULTS = None
_CACHE = {}


def const_layout(n_b=NB, n_sc=NSC):
    """Column offsets inside the single bf16 constants tensor [128, CW]."""
    off = {}
    off["wt"] = 0  # [p, kc*H + h] = W_h[h, kc*128+p]
    off["vemb"] = off["wt"] + NKC * H  # [p, (hc*n_sc+sc)*n_sc + m]
    off["wst"] = off["vemb"] + NHC * n_sc * n_sc  # [p, ec*H + hc*128 + h]
    off["dect"] = off["wst"] + NEC * H  # [p, ec*n_b + b]   (per-core)
    off["whb"] = off["dect"] + NEC * n_b  # row 0 only: [0, hc*128+h]
    off["ones"] = off["whb"] + H  # row 0 only: [0, 0:n_b] = 1.0
    off["_width"] = off["ones"] + n_b
    return off


def build_bass(n_b=NB, n_sc=NSC):
    import concourse.bass as bass
    import concourse.tile as tile
    from concourse import mybir
    from contextlib import ExitStack

    f32 = mybir.dt.float32
    bf16 = mybir.dt.bfloat16
    Tanh = mybir.ActivationFunctionType.Tanh
    Exp = mybir.ActivationFunctionType.Exp
    mult = mybir.AluOpType.mult
    add = mybir.AluOpType.add

    ncols = n_b * n_sc * 512
    lay = const_layout(n_b, n_sc)
    CW = lay["_width"]
    FW = n_b * 512 + n_sc  # f32 tensor: mask then ones columns

    fp8 = mybir.dt.float8e4
    DR = mybir.MatmulPerfMode.DoubleRow
    enc_dt = fp8 if USE_FP8 else bf16

    nc = bass.Bass()
    enc_ext = nc.declare_dram_parameter("encT", [128, NKC, ncols], enc_dt, False)
    cst_ext = nc.declare_dram_parameter("consts", [128, CW], bf16, False)
    f32_ext = nc.declare_dram_parameter("f32c", [n_sc, FW], f32, False)
    if USE_FP8:
        w8_ext = nc.declare_dram_parameter("w8", [128, NKC // 2, 2, NHC, 128], fp8, False)
    out_ext = nc.declare_dram_parameter("out", [n_b, n_sc, 512], f32, True)

    with ExitStack() as ctx:
        tc = ctx.enter_context(tile.TileContext(nc))
        const = ctx.enter_context(tc.tile_pool(name="const", bufs=1))
        encp = ctx.enter_context(tc.tile_pool(name="enc", bufs=4))
        xp = ctx.enter_context(tc.tile_pool(name="x", bufs=8))
        bp = ctx.enter_context(tc.tile_pool(name="bt", bufs=8))
        psmm = ctx.enter_context(tc.tile_pool(name="psmm", bufs=3, space="PSUM"))
        pssc = ctx.enter_context(tc.tile_pool(name="pssc", bufs=2, space="PSUM"))
        psb = ctx.enter_context(tc.tile_pool(name="psb", bufs=2, space="PSUM"))
        psdec_p = ctx.enter_context(tc.tile_pool(name="psdec", bufs=1, space="PSUM"))

        cst = const.tile([128, CW], bf16, tag="cst")
        nc.sync.dma_start(cst[:], cst_ext[:])
        if USE_FP8:
            w8_sb = const.tile([128, NKC // 2, 2, NHC, 128], fp8, tag="w8")
            nc.sync.dma_start(w8_sb[:], w8_ext[:])
        f32_0 = const.tile([n_sc, FW], f32, tag="f32_0")
        nc.sync.dma_start(f32_0[:], f32_ext[:])
        # re-home the f32 constants onto ScalarE: the VectorE reduce then waits
        # on the ScalarE semaphore alone (which also covers exp outputs)
        f32c = const.tile([n_sc, FW], f32, tag="f32c")
        nc.scalar.copy(f32c[:], f32_0[:])

        def cs(name, lo, width):
            return cst[:, lay[name] + lo : lay[name] + lo + width]

        # Preload the exp_and_others ACT table set (tanh+exp share it) so the
        # implicit table-load pseudo doesn't ride on a hot-loop instruction.
        warm = const.tile([1, 3], f32, tag="warm")
        nc.scalar.activation(warm[:, 0:1], f32_0[0:1, 0:1], Tanh)
        nc.scalar.activation(warm[:, 1:2], f32_0[0:1, 0:1], Exp)

        # dec projection: bias[h, b] = sum_e W_s[h,e] dec[b,e] + (b_h+b_s)[h]
        psdec = psdec_p.tile([128, NHC * n_b], f32, tag="psdec")
        for hc in range(NHC):
            for ec in range(NEC):
                nc.tensor.matmul(
                    psdec[:, hc * n_b : (hc + 1) * n_b],
                    cs("wst", ec * H + hc * 128, 128),
                    cs("dect", ec * n_b, n_b),
                    start=(ec == 0),
                    stop=False,
                )
            nc.tensor.matmul(
                psdec[:, hc * n_b : (hc + 1) * n_b],
                cst[0:1, lay["whb"] + hc * 128 : lay["whb"] + (hc + 1) * 128],
                cst[0:1, lay["ones"] : lay["ones"] + n_b],
                start=False,
                stop=True,
            )
        # evacuate on VectorE, then touch once on ScalarE: the hot-loop tanh
        # then needs only its PE wait (hw instructions hold a single wait)
        bias_sb = const.tile([128, NHC * n_b], f32, tag="bias")
        nc.vector.tensor_copy(bias_sb[:], psdec[:])
        nc.scalar.copy(warm[:, 2:3], bias_sb[0:1, 0:1])

        for b in range(n_b):
            ps_sc = pssc.tile([n_sc, 512], f32, tag="ps_sc")
            for sc in range(n_sc):
                c0 = (b * n_sc + sc) * 512
                et = encp.tile([128, NKC, 512], enc_dt, tag="et")
                nc.sync.dma_start(et[:], enc_ext[:, :, c0 : c0 + 512])
                for hc in range(NHC):
                    ps = psmm.tile([128, 512], f32, tag="ps")
                    if USE_FP8:
                        for kc2 in range(NKC // 2):
                            nc.tensor.matmul(
                                ps[:],
                                w8_sb[:, kc2, :, hc, :],
                                et[:, 2 * kc2 : 2 * kc2 + 2, :],
                                start=(kc2 == 0),
                                stop=(kc2 == NKC // 2 - 1),
                                perf_mode=DR,
                            )
                    else:
                        for kc in range(NKC):
                            nc.tensor.matmul(
                                ps[:],
                                cs("wt", kc * H + hc * 128, 128),
                                et[:, kc, :],
                                start=(kc == 0),
                                stop=(kc == NKC - 1),
                            )
                    xt = xp.tile([128, 512], bf16, tag="xt")
                    nc.scalar.activation(
                        xt[:],
                        ps[:],
                        Tanh,
                        bias=bias_sb[:, hc * n_b + b : hc * n_b + b + 1],
                    )
                    nc.tensor.matmul(
                        ps_sc[:],
                        cs("vemb", (hc * n_sc + sc) * n_sc, n_sc),
                        xt[:],
                        start=(sc == 0 and hc == 0),
                        stop=(sc == n_sc - 1 and hc == NHC - 1),
                    )
                # ScalarE observes its own newest tick so recycled xt slots
                # never add a second (same-engine WAW) wait to a later tanh
                nc.scalar.copy(warm[:, 2:3], xt[0:1, 0:1])
            # batch epilogue: masked softmax over the n_sc*512 scores of batch b
            u8 = bp.tile([n_sc, 512], f32, tag="u8")
            nc.scalar.activation(u8[:], ps_sc[:], Exp)
            uw = bp.tile([n_sc, 512], f32, tag="uw")
            part = bp.tile([n_sc, 1], f32, tag="part")
            nc.vector.tensor_mul(uw[:], u8[:], f32c[:, b * 512 : (b + 1) * 512])
            nc.vector.tensor_reduce(
                part[:], uw[:], axis=mybir.AxisListType.X, op=add
            )
            tot_ps = psb.tile([n_sc, 1], f32, tag="tot")
            nc.tensor.matmul(tot_ps[:], f32c[:, n_b * 512 : n_b * 512 + n_sc], part[:])
            recip = bp.tile([n_sc, 1], f32, tag="recip")
            nc.vector.reciprocal(recip[:], tot_ps[:])
            probs = bp.tile([n_sc, 512], f32, tag="probs")
            nc.vector.tensor_scalar_mul(probs[:], uw[:], recip[:])
            nc.sync.dma_start(out_ext[b], probs[:])

    return nc


def legalize_single_wait(nc):
    """The walrus in this container accepts at most ONE sync wait per
    instruction and cannot encode EVENT_SEMAPHORE_RANGE_CLEAR.  Split excess
    waits onto single-wait NOPs, and replace the tile-exit range clear with
    per-semaphore decrements of the statically known final values."""
    import concourse.mybir as mybir
    import bass_rust

    m = nc.m
    totals = {}
    names = {}
    for fn in m.functions:
        for blk in fn.blocks:
            for inst in blk.instructions:
                si = getattr(inst, "sync_info", None)
                if not si:
                    continue
                for u in si.on_update or []:
                    if u.sync_type != "semaphore":
                        continue
                    v = u.update_value if u.update_value is not None else 1
                    if u.update_mode in ("sem-inc", "sem-add-imm"):
                        totals[u.id] = totals.get(u.id, 0) + v
                    elif u.update_mode in ("sem-dec", "sem-sub-imm"):
                        totals[u.id] = totals.get(u.id, 0) - v
                    names[u.id] = u.ant_name

    nid = [0]

    def mk_nop(engine, wait):
        nid[0] += 1
        nop = mybir.InstNoOp(name=f"I-lsw-{nid[0]}", engine=engine, ins=[], outs=[])
        nop.sync_info = bass_rust.SyncInfo(on_wait=[wait], on_update=[])
        return nop

    def mk_dec(engine, sem_id, value):
        nid[0] += 1
        es = mybir.InstEventSemaphore(
            name=f"I-lsc-{nid[0]}", engine=engine, ins=[], outs=[]
        )
        u = bass_rust.SyncUpdate(
            sync_type="semaphore",
            id=sem_id,
            ant_name=names.get(sem_id, f"sem{sem_id}"),
            update_mode="sem-sub-imm",
            update_value=value,
            update_reg=None,
        )
        es.sync_info = bass_rust.SyncInfo(on_wait=[], on_update=[u])
        return es

    for fn in m.functions:
        for blk in fn.blocks:
            out = []
            for inst in blk.instructions:
                if (
                    isinstance(inst, mybir.InstISA)
                    and getattr(inst, "isa_opcode", None) == 176
                ):
                    first = getattr(inst, "range_first", None)
                    last = getattr(inst, "range_last", None)
                    if first is None:
                        d = inst.concise()
                        import re

                        first = int(re.search(r"range_first=(\d+)", d).group(1))
                        last = int(re.search(r"range_last=(\d+)", d).group(1))
                    for sem_id in range(first, last + 1):
                        v = totals.get(sem_id, 0)
                        if v > 0:
                            out.append(mk_dec(inst.engine, sem_id, v))
                    continue
                si = getattr(inst, "sync_info", None)
                waits = list(si.on_wait) if si and si.on_wait else []
                if len(waits) > 1:
                    for w in waits[:-1]:
                        out.append(mk_nop(inst.engine, w))
                    inst.sync_info = bass_rust.SyncInfo(
                        on_wait=[waits[-1]], on_update=list(si.on_update or [])
                    )
                out.append(inst)
            blk.instructions = out


def prep_w8(W_h_w):
    """DoubleRow fp8 weights: w8[p, kc2, i, hc, h] = W_h[hc*128+h, kc2*256+i*128+p]."""
    Wh = np.asarray(W_h_w, np.float32)  # [H, E]
    return np.ascontiguousarray(
        Wh.T.reshape(NKC // 2, 2, 128, NHC, 128).transpose(2, 0, 1, 3, 4)
    ).astype(FP8)


def prep_shared(W_h_w, W_h_b, W_s_w, W_s_b, v_w, n_b=NB, n_sc=NSC):
    """The shared part of the constants tensor (zero where dect goes)."""
    lay = const_layout(n_b, n_sc)
    Wh = np.asarray(W_h_w, np.float32)  # [H, E]
    Ws = np.asarray(W_s_w, np.float32)  # [H, H]
    v = np.asarray(v_w, np.float32).reshape(H)

    cst = np.zeros((128, lay["_width"]), np.float32)
    # wt[p, kc*H + h] = Wh[h, kc*128+p]
    cst[:, lay["wt"] : lay["wt"] + NKC * H] = (
        Wh.T.reshape(NKC, 128, H).transpose(1, 0, 2).reshape(128, NKC * H)
    )
    # vemb[p, (hc*n_sc+sc)*n_sc + m] = (m==sc) * v[hc*128+p]
    vr = v.reshape(NHC, 128).T  # [p, hc]
    vemb = np.zeros((128, NHC, n_sc, n_sc), np.float32)
    for scm in range(n_sc):
        vemb[:, :, scm, scm] = vr
    cst[:, lay["vemb"] : lay["vemb"] + NHC * n_sc * n_sc] = vemb.reshape(128, -1)
    # wst[p, ec*H + hc*128 + h] = Ws[hc*128+h, ec*128+p]
    cst[:, lay["wst"] : lay["wst"] + NEC * H] = (
        Ws.T.reshape(NEC, 128, NHC, 128).transpose(1, 0, 2, 3).reshape(128, NEC * H)
    )
    # whb row 0: combined bias, ones row 0
    cst[0, lay["whb"] : lay["whb"] + H] = np.asarray(W_h_b, np.float32) + np.asarray(
        W_s_b, np.float32
    )
    cst[0, lay["ones"] : lay["ones"] + n_b] = 1.0
    return cst


def prep_core(cst_shared, dec_shard, enc_shard, mask_shard, n_b=NB, n_sc=NSC):
    """Per-core input map (layout transforms + bf16 cast only)."""
    lay = const_layout(n_b, n_sc)
    ncols = n_b * n_sc * 512
    dt = FP8 if USE_FP8 else BF16
    rows = np.asarray(enc_shard, np.float32).reshape(ncols, E).astype(dt)
    # encT[p, kc, col] = enc[col, kc*128+p]
    encT = np.ascontiguousarray(rows.T.reshape(NKC, 128, ncols).transpose(1, 0, 2))
    cst = cst_shared.copy()
    # dect[p, ec*n_b + b] = dec[b, ec*128+p]
    cst[:, lay["dect"] : lay["dect"] + NEC * n_b] = (
        np.asarray(dec_shard, np.float32)
        .T.reshape(NEC, 128, n_b)
        .transpose(1, 0, 2)
        .reshape(128, NEC * n_b)
    )
    # f32c: mask columns then ones columns
    f32c = np.ones((n_sc, n_b * 512 + n_sc), np.float32)
    f32c[:, : n_b * 512] = (
        np.asarray(mask_shard)
        .astype(np.float32)
        .reshape(n_b, n_sc, 512)
        .transpose(1, 0, 2)
        .reshape(n_sc, n_b * 512)
    )
    return {"encT": encT, "consts": cst.astype(BF16), "f32c": f32c}


def kernel(decoder_output, encoder_output, mask, W_h_w, W_h_b, W_s_w, W_s_b, v_w):
    global LAST_EXEC_NS, LAST_RESULTS
    from concourse.bass_utils import run_bass_kernel_spmd

    if "nc" not in _CACHE:
        nc0 = build_bass()
        nc0.finalize()
        legalize_single_wait(nc0)
        _CACHE["nc"] = nc0
    nc = _CACHE["nc"]

    shared = prep_shared(W_h_w, W_h_b, W_s_w, W_s_b, v_w)
    w8 = prep_w8(W_h_w) if USE_FP8 else None
    in_maps = []
    for c in range(NCORES):
        sl = slice(c * NB, (c + 1) * NB)
        m = prep_core(shared, decoder_output[sl], encoder_output[sl], mask[sl])
        if USE_FP8:
            m["w8"] = w8
        in_maps.append(m)

    res = run_bass_kernel_spmd(nc, in_maps, core_ids=list(range(NCORES)), trace=TRACE)
    if TRACE:
        LAST_EXEC_NS = res.exec_time_ns
        LAST_RESULTS = res
    out = np.concatenate(
        [
            np.asarray(res.results[c]["out"], np.float32).reshape(NB, S)
            for c in range(NCORES)
        ],
        axis=0,
    )
    return out


# revision 14
# speedup vs baseline: 1.7451x; 1.1080x over previous
"""Bahdanau-attention kernel for Trainium2 (8 NeuronCores, SPMD data-parallel).

Computes softmax(mask(v . tanh(enc @ W_h^T + dec @ W_s^T + b_h + b_s))) for
B=64, S=4096, H=512, E=1024.  Sharded data-parallel over batch: 8 batches per
core, weights replicated.  All heavy compute (the [32768,1024]@[1024,512]
projection per core) runs on the TensorEngine in bf16; softmax runs on
ScalarE/VectorE without a max-subtraction (|scores| <= sum|v| ~ 11.3, so exp
cannot overflow).
"""

import sys

import numpy as np

try:
    import concourse.bass as bass  # noqa: F401
except ImportError:  # pragma: no cover
    sys.path.insert(0, "/opt/trn_rl_repo")
    import concourse.bass as bass  # noqa: F401

import ml_dtypes

BF16 = ml_dtypes.bfloat16
FP8 = ml_dtypes.float8_e4m3
USE_FP8 = True
VMM_FP8 = True

B, S, H = 64, 4096, 512
E = 2 * H  # 1024
NCORES = 8
NB = B // NCORES  # 8 local batches per core
NSC = S // 512  # 8 sequence chunks of 512 per batch
NKC = E // 128  # 8 contraction chunks
NHC = H // 128  # 4 h chunks
NEC = 4  # contraction chunks for the dec projection (512/128)

TRACE = False
LAST_EXEC_NS = None
LAST_RESULTS = None
_CACHE = {}


def const_layout(n_b=NB, n_sc=NSC):
    """Column offsets inside the single bf16 constants tensor [128, CW]."""
    off = {}
    off["wt"] = 0  # [p, kc*H + h] = W_h[h, kc*128+p] (bf16 fallback only)
    off["vemb"] = off["wt"] + (0 if USE_FP8 else NKC * H)
    off["wst"] = off["vemb"] + NHC * n_sc * n_sc  # [p, ec*H + hc*128 + h]
    off["dect"] = off["wst"] + NEC * H  # [p, ec*n_b + b]   (per-core)
    off["whb"] = off["dect"] + NEC * n_b  # row 0 only: [0, hc*128+h]
    off["ones"] = off["whb"] + H  # row 0 only: [0, 0:n_b] = 1.0
    off["_width"] = off["ones"] + n_b
    return off


def build_bass(n_b=NB, n_sc=NSC):
    import concourse.bass as bass
    import concourse.tile as tile
    from concourse import mybir
    from contextlib import ExitStack

    f32 = mybir.dt.float32
    bf16 = mybir.dt.bfloat16
    Tanh = mybir.ActivationFunctionType.Tanh
    Exp = mybir.ActivationFunctionType.Exp
    mult = mybir.AluOpType.mult
    add = mybir.AluOpType.add

    ncols = n_b * n_sc * 512
    lay = const_layout(n_b, n_sc)
    CW = lay["_width"]
    FW = n_b * 512 + n_sc  # f32 tensor: mask then ones columns

    fp8 = mybir.dt.float8e4
    DR = mybir.MatmulPerfMode.DoubleRow
    enc_dt = fp8 if USE_FP8 else bf16

    nc = bass.Bass()
    enc_ext = nc.declare_dram_parameter("encT", [128, NKC, ncols], enc_dt, False)
    cst_ext = nc.declare_dram_parameter("consts", [128, CW], bf16, False)
    f32_ext = nc.declare_dram_parameter("f32c", [n_sc, FW], f32, False)
    if USE_FP8:
        w8_ext = nc.declare_dram_parameter("w8", [128, NKC // 2, 2, NHC, 128], fp8, False)
    if USE_FP8 and VMM_FP8:
        v8_ext = nc.declare_dram_parameter(
            "v8", [128, NHC // 2, n_sc, 2, 16], fp8, False
        )
    out_ext = nc.declare_dram_parameter("out", [n_b, n_sc, 512], f32, True)

    with ExitStack() as ctx:
        tc = ctx.enter_context(tile.TileContext(nc))
        const = ctx.enter_context(tc.tile_pool(name="const", bufs=1))
        encp = ctx.enter_context(tc.tile_pool(name="enc", bufs=4))
        xp = ctx.enter_context(tc.tile_pool(name="x", bufs=8))
        bp = ctx.enter_context(tc.tile_pool(name="bt", bufs=8))
        psmm = ctx.enter_context(tc.tile_pool(name="psmm", bufs=3, space="PSUM"))
        pssc = ctx.enter_context(tc.tile_pool(name="pssc", bufs=2, space="PSUM"))
        psb = ctx.enter_context(tc.tile_pool(name="psb", bufs=2, space="PSUM"))
        psdec_p = ctx.enter_context(tc.tile_pool(name="psdec", bufs=1, space="PSUM"))

        cst = const.tile([128, CW], bf16, tag="cst")
        nc.sync.dma_start(cst[:], cst_ext[:])
        if USE_FP8:
            w8_sb = const.tile([128, NKC // 2, 2, NHC, 128], fp8, tag="w8")
            nc.sync.dma_start(w8_sb[:], w8_ext[:])
        if USE_FP8 and VMM_FP8:
            v8_sb = const.tile([128, NHC // 2, n_sc, 2, 16], fp8, tag="v8")
            nc.sync.dma_start(v8_sb[:], v8_ext[:])
        f32_0 = const.tile([n_sc, FW], f32, tag="f32_0")
        nc.sync.dma_start(f32_0[:], f32_ext[:])
        # re-home the f32 constants onto ScalarE: the VectorE reduce then waits
        # on the ScalarE semaphore alone (which also covers exp outputs)
        f32c = const.tile([n_sc, FW], f32, tag="f32c")
        nc.scalar.copy(f32c[:], f32_0[:])

        def cs(name, lo, width):
            return cst[:, lay[name] + lo : lay[name] + lo + width]

        # Preload the exp_and_others ACT table set (tanh+exp share it) so the
        # implicit table-load pseudo doesn't ride on a hot-loop instruction.
        warm = const.tile([1, 3], f32, tag="warm")
        nc.scalar.activation(warm[:, 0:1], f32_0[0:1, 0:1], Tanh)
        nc.scalar.activation(warm[:, 1:2], f32_0[0:1, 0:1], Exp)

        # dec projection: bias[h, b] = sum_e W_s[h,e] dec[b,e] + (b_h+b_s)[h]
        psdec = psdec_p.tile([128, NHC * n_b], f32, tag="psdec")
        for hc in range(NHC):
            for ec in range(NEC):
                nc.tensor.matmul(
                    psdec[:, hc * n_b : (hc + 1) * n_b],
                    cs("wst", ec * H + hc * 128, 128),
                    cs("dect", ec * n_b, n_b),
                    start=(ec == 0),
                    stop=False,
                )
            nc.tensor.matmul(
                psdec[:, hc * n_b : (hc + 1) * n_b],
                cst[0:1, lay["whb"] + hc * 128 : lay["whb"] + (hc + 1) * 128],
                cst[0:1, lay["ones"] : lay["ones"] + n_b],
                start=False,
                stop=True,
            )
        # evacuate on VectorE, then touch once on ScalarE: the hot-loop tanh
        # then needs only its PE wait (hw instructions hold a single wait)
        bias_sb = const.tile([128, NHC * n_b], f32, tag="bias")
        nc.vector.tensor_copy(bias_sb[:], psdec[:])
        nc.scalar.copy(warm[:, 2:3], bias_sb[0:1, 0:1])

        vmm_dr = USE_FP8 and VMM_FP8
        for b in range(n_b):
            ps_sc = pssc.tile([16 if vmm_dr else n_sc, 512], f32, tag="ps_sc")
            for sc in range(n_sc):
                c0 = (b * n_sc + sc) * 512
                et = encp.tile([128, NKC, 512], enc_dt, tag="et")
                nc.sync.dma_start(et[:], enc_ext[:, :, c0 : c0 + 512])
                if vmm_dr:
                    xt4 = xp.tile([128, NHC, 512], fp8, tag="xt")
                for hc in range(NHC):
                    ps = psmm.tile([128, 512], f32, tag="ps")
                    if USE_FP8:
                        for kc2 in range(NKC // 2):
                            nc.tensor.matmul(
                                ps[:],
                                w8_sb[:, kc2, :, hc, :],
                                et[:, 2 * kc2 : 2 * kc2 + 2, :],
                                start=(kc2 == 0),
                                stop=(kc2 == NKC // 2 - 1),
                                perf_mode=DR,
                            )
                    else:
                        for kc in range(NKC):
                            nc.tensor.matmul(
                                ps[:],
                                cs("wt", kc * H + hc * 128, 128),
                                et[:, kc, :],
                                start=(kc == 0),
                                stop=(kc == NKC - 1),
                            )
                    if vmm_dr:
                        nc.scalar.activation(
                            xt4[:, hc, :],
                            ps[:],
                            Tanh,
                            bias=bias_sb[:, hc * n_b + b : hc * n_b + b + 1],
                        )
                    else:
                        xt = xp.tile([128, 512], bf16, tag="xt")
                        nc.scalar.activation(
                            xt[:],
                            ps[:],
                            Tanh,
                            bias=bias_sb[:, hc * n_b + b : hc * n_b + b + 1],
                        )
                        nc.tensor.matmul(
                            ps_sc[:],
                            cs("vemb", (hc * n_sc + sc) * n_sc, n_sc),
                            xt[:],
                            start=(sc == 0 and hc == 0),
                            stop=(sc == n_sc - 1 and hc == NHC - 1),
                        )
                if vmm_dr:
                    for j in range(NHC // 2):
                        nc.tensor.matmul(
                            ps_sc[:],
                            v8_sb[:, j, sc, :, :],
                            xt4[:, 2 * j : 2 * j + 2, :],
                            start=(sc == 0 and j == 0),
                            stop=(sc == n_sc - 1 and j == NHC // 2 - 1),
                            perf_mode=DR,
                        )
                # ScalarE observes its own newest tick so recycled xt slots
                # never add a second (same-engine WAW) wait to a later tanh
                if vmm_dr:
                    nc.scalar.copy(warm[:, 2:3], xt4[0:1, 0:1, 0:1])
                else:
                    nc.scalar.copy(warm[:, 2:3], xt[0:1, 0:1])
            # batch epilogue: masked softmax over the n_sc*512 scores of batch b
            u8 = bp.tile([n_sc, 512], f32, tag="u8")
            nc.scalar.activation(u8[:], ps_sc[: n_sc, :], Exp)
            uw = bp.tile([n_sc, 512], f32, tag="uw")
            part = bp.tile([n_sc, 1], f32, tag="part")
            nc.vector.tensor_mul(uw[:], u8[:], f32c[:, b * 512 : (b + 1) * 512])
            nc.vector.tensor_reduce(
                part[:], uw[:], axis=mybir.AxisListType.X, op=add
            )
            tot_ps = psb.tile([n_sc, 1], f32, tag="tot")
            nc.tensor.matmul(tot_ps[:], f32c[:, n_b * 512 : n_b * 512 + n_sc], part[:])
            recip = bp.tile([n_sc, 1], f32, tag="recip")
            nc.vector.reciprocal(recip[:], tot_ps[:])
            probs = bp.tile([n_sc, 512], f32, tag="probs")
            nc.vector.tensor_scalar_mul(probs[:], uw[:], recip[:])
            nc.sync.dma_start(out_ext[b], probs[:])

    return nc


def legalize_single_wait(nc):
    """The walrus in this container accepts at most ONE sync wait per
    instruction and cannot encode EVENT_SEMAPHORE_RANGE_CLEAR.  Split excess
    waits onto single-wait NOPs, and replace the tile-exit range clear with
    per-semaphore decrements of the statically known final values."""
    import concourse.mybir as mybir
    import bass_rust

    m = nc.m
    totals = {}
    names = {}
    for fn in m.functions:
        for blk in fn.blocks:
            for inst in blk.instructions:
                si = getattr(inst, "sync_info", None)
                if not si:
                    continue
                for u in si.on_update or []:
                    if u.sync_type != "semaphore":
                        continue
                    v = u.update_value if u.update_value is not None else 1
                    if u.update_mode in ("sem-inc", "sem-add-imm"):
                        totals[u.id] = totals.get(u.id, 0) + v
                    elif u.update_mode in ("sem-dec", "sem-sub-imm"):
                        totals[u.id] = totals.get(u.id, 0) - v
                    names[u.id] = u.ant_name

    nid = [0]

    def mk_nop(engine, wait):
        nid[0] += 1
        nop = mybir.InstNoOp(name=f"I-lsw-{nid[0]}", engine=engine, ins=[], outs=[])
        nop.sync_info = bass_rust.SyncInfo(on_wait=[wait], on_update=[])
        return nop

    def mk_dec(engine, sem_id, value):
        nid[0] += 1
        es = mybir.InstEventSemaphore(
            name=f"I-lsc-{nid[0]}", engine=engine, ins=[], outs=[]
        )
        u = bass_rust.SyncUpdate(
            sync_type="semaphore",
            id=sem_id,
            ant_name=names.get(sem_id, f"sem{sem_id}"),
            update_mode="sem-sub-imm",
            update_value=value,
            update_reg=None,
        )
        es.sync_info = bass_rust.SyncInfo(on_wait=[], on_update=[u])
        return es

    for fn in m.functions:
        for blk in fn.blocks:
            out = []
            for inst in blk.instructions:
                if (
                    isinstance(inst, mybir.InstISA)
                    and getattr(inst, "isa_opcode", None) == 176
                ):
                    first = getattr(inst, "range_first", None)
                    last = getattr(inst, "range_last", None)
                    if first is None:
                        d = inst.concise()
                        import re

                        first = int(re.search(r"range_first=(\d+)", d).group(1))
                        last = int(re.search(r"range_last=(\d+)", d).group(1))
                    for sem_id in range(first, last + 1):
                        v = totals.get(sem_id, 0)
                        if v > 0:
                            out.append(mk_dec(inst.engine, sem_id, v))
                    continue
                si = getattr(inst, "sync_info", None)
                waits = list(si.on_wait) if si and si.on_wait else []
                if len(waits) > 1:
                    for w in waits[:-1]:
                        out.append(mk_nop(inst.engine, w))
                    inst.sync_info = bass_rust.SyncInfo(
                        on_wait=[waits[-1]], on_update=list(si.on_update or [])
                    )
                out.append(inst)
            blk.instructions = out


def prep_w8(W_h_w):
    """DoubleRow fp8 weights: w8[p, kc2, i, hc, h] = W_h[hc*128+h, kc2*256+i*128+p]."""
    Wh = np.asarray(W_h_w, np.float32)  # [H, E]
    return np.ascontiguousarray(
        Wh.T.reshape(NKC // 2, 2, 128, NHC, 128).transpose(2, 0, 1, 3, 4)
    ).astype(FP8)


def prep_v8(v_w, n_sc=NSC):
    """DoubleRow fp8 v embedding: v8[p, j, sc, i, m] = (m==sc) v[(2j+i)*128+p]."""
    v = np.asarray(v_w, np.float32).reshape(H)
    v8 = np.zeros((128, NHC // 2, n_sc, 2, 16), np.float32)
    vr = v.reshape(NHC // 2, 2, 128)  # [j, i, p]
    for sc in range(n_sc):
        v8[:, :, sc, :, sc] = vr.transpose(2, 0, 1)
    return v8.astype(FP8)


def prep_shared(W_h_w, W_h_b, W_s_w, W_s_b, v_w, n_b=NB, n_sc=NSC):
    """The shared part of the constants tensor (zero where dect goes)."""
    lay = const_layout(n_b, n_sc)
    Wh = np.asarray(W_h_w, np.float32)  # [H, E]
    Ws = np.asarray(W_s_w, np.float32)  # [H, H]
    v = np.asarray(v_w, np.float32).reshape(H)

    cst = np.zeros((128, lay["_width"]), np.float32)
    # wt[p, kc*H + h] = Wh[h, kc*128+p]
    if not USE_FP8:
        cst[:, lay["wt"] : lay["wt"] + NKC * H] = (
            Wh.T.reshape(NKC, 128, H).transpose(1, 0, 2).reshape(128, NKC * H)
        )
    # vemb[p, (hc*n_sc+sc)*n_sc + m] = (m==sc) * v[hc*128+p]
    vr = v.reshape(NHC, 128).T  # [p, hc]
    vemb = np.zeros((128, NHC, n_sc, n_sc), np.float32)
    for scm in range(n_sc):
        vemb[:, :, scm, scm] = vr
    cst[:, lay["vemb"] : lay["vemb"] + NHC * n_sc * n_sc] = vemb.reshape(128, -1)
    # wst[p, ec*H + hc*128 + h] = Ws[hc*128+h, ec*128+p]
    cst[:, lay["wst"] : lay["wst"] + NEC * H] = (
        Ws.T.reshape(NEC, 128, NHC, 128).transpose(1, 0, 2, 3).reshape(128, NEC * H)
    )
    # whb row 0: combined bias, ones row 0
    cst[0, lay["whb"] : lay["whb"] + H] = np.asarray(W_h_b, np.float32) + np.asarray(
        W_s_b, np.float32
    )
    cst[0, lay["ones"] : lay["ones"] + n_b] = 1.0
    return cst


def prep_core(cst_shared, dec_shard, enc_shard, mask_shard, n_b=NB, n_sc=NSC):
    """Per-core input map (layout transforms + bf16 cast only)."""
    lay = const_layout(n_b, n_sc)
    ncols = n_b * n_sc * 512
    dt = FP8 if USE_FP8 else BF16
    rows = np.asarray(enc_shard, np.float32).reshape(ncols, E).astype(dt)
    # encT[p, kc, col] = enc[col, kc*128+p]
    encT = np.ascontiguousarray(rows.T.reshape(NKC, 128, ncols).transpose(1, 0, 2))
    cst = cst_shared.copy()
    # dect[p, ec*n_b + b] = dec[b, ec*128+p]
    cst[:, lay["dect"] : lay["dect"] + NEC * n_b] = (
        np.asarray(dec_shard, np.float32)
        .T.reshape(NEC, 128, n_b)
        .transpose(1, 0, 2)
        .reshape(128, NEC * n_b)
    )
    # f32c: mask columns then ones columns
    f32c = np.ones((n_sc, n_b * 512 + n_sc), np.float32)
    f32c[:, : n_b * 512] = (
        np.asarray(mask_shard)
        .astype(np.float32)
        .reshape(n_b, n_sc, 512)
        .transpose(1, 0, 2)
        .reshape(n_sc, n_b * 512)
    )
    return {"encT": encT, "consts": cst.astype(BF16), "f32c": f32c}


def kernel(decoder_output, encoder_output, mask, W_h_w, W_h_b, W_s_w, W_s_b, v_w):
    global LAST_EXEC_NS, LAST_RESULTS
    from concourse.bass_utils import run_bass_kernel_spmd

    if "nc" not in _CACHE:
        nc0 = build_bass()
        nc0.finalize()
        legalize_single_wait(nc0)
        _CACHE["nc"] = nc0
    nc = _CACHE["nc"]

    shared = prep_shared(W_h_w, W_h_b, W_s_w, W_s_b, v_w)
    w8 = prep_w8(W_h_w) if USE_FP8 else None
    v8 = prep_v8(v_w) if (USE_FP8 and VMM_FP8) else None
    in_maps = []
    for c in range(NCORES):
        sl = slice(c * NB, (c + 1) * NB)
        m = prep_core(shared, decoder_output[sl], encoder_output[sl], mask[sl])
        if USE_FP8:
            m["w8"] = w8
        if v8 is not None:
            m["v8"] = v8
        in_maps.append(m)

    res = run_bass_kernel_spmd(nc, in_maps, core_ids=list(range(NCORES)), trace=TRACE)
    if TRACE:
        LAST_EXEC_NS = res.exec_time_ns
        LAST_RESULTS = res
    out = np.concatenate(
        [
            np.asarray(res.results[c]["out"], np.float32).reshape(NB, S)
            for c in range(NCORES)
        ],
        axis=0,
    )
    return out


# revision 17
# speedup vs baseline: 2.0739x; 1.1884x over previous
"""Bahdanau-attention kernel for Trainium2 (8 NeuronCores, SPMD data-parallel).

Computes softmax(mask(v . tanh(enc @ W_h^T + dec @ W_s^T + b_h + b_s))) for
B=64, S=4096, H=512, E=1024.  Sharded data-parallel over batch: 8 batches per
core, weights replicated.  All heavy compute (the [32768,1024]@[1024,512]
projection per core) runs on the TensorEngine in bf16; softmax runs on
ScalarE/VectorE without a max-subtraction (|scores| <= sum|v| ~ 11.3, so exp
cannot overflow).
"""

import sys

import numpy as np

try:
    import concourse.bass as bass  # noqa: F401
except ImportError:  # pragma: no cover
    sys.path.insert(0, "/opt/trn_rl_repo")
    import concourse.bass as bass  # noqa: F401

import ml_dtypes

BF16 = ml_dtypes.bfloat16
FP8 = ml_dtypes.float8_e4m3
USE_FP8 = True
VMM_FP8 = True

B, S, H = 64, 4096, 512
E = 2 * H  # 1024
NCORES = 8
NB = B // NCORES  # 8 local batches per core
NSC = S // 512  # 8 sequence chunks of 512 per batch
NKC = E // 128  # 8 contraction chunks
NHC = H // 128  # 4 h chunks
NEC = 4  # contraction chunks for the dec projection (512/128)

TRACE = False
LAST_EXEC_NS = None
LAST_RESULTS = None
_CACHE = {}


def const_layout(n_b=NB, n_sc=NSC):
    """Column offsets inside the single bf16 constants tensor [128, CW]."""
    off = {}
    off["wt"] = 0  # [p, kc*H + h] = W_h[h, kc*128+p] (bf16 fallback only)
    off["vemb"] = off["wt"] + (0 if USE_FP8 else NKC * H)
    off["wst"] = off["vemb"] + NHC * n_sc * n_sc  # [p, ec*H + hc*128 + h]
    off["dect"] = off["wst"] + NEC * H  # [p, ec*n_b + b]   (per-core)
    off["whb"] = off["dect"] + NEC * n_b  # row 0 only: [0, hc*128+h]
    off["ones"] = off["whb"] + H  # row 0 only: [0, 0:n_b] = 1.0
    off["_width"] = off["ones"] + n_b
    return off


def build_bass(n_b=NB, n_sc=NSC):
    import concourse.bass as bass
    import concourse.tile as tile
    from concourse import mybir
    from contextlib import ExitStack

    f32 = mybir.dt.float32
    bf16 = mybir.dt.bfloat16
    Tanh = mybir.ActivationFunctionType.Tanh
    Exp = mybir.ActivationFunctionType.Exp
    mult = mybir.AluOpType.mult
    add = mybir.AluOpType.add

    ncols = n_b * n_sc * 512
    lay = const_layout(n_b, n_sc)
    CW = lay["_width"]
    FW = n_b * 512 + n_sc  # f32 tensor: mask then ones columns

    fp8 = mybir.dt.float8e4
    DR = mybir.MatmulPerfMode.DoubleRow
    enc_dt = fp8 if USE_FP8 else bf16

    nc = bass.Bass()
    enc_ext = nc.declare_dram_parameter("encT", [128, NKC, ncols], enc_dt, False)
    cst_ext = nc.declare_dram_parameter("consts", [128, CW], bf16, False)
    f32_ext = nc.declare_dram_parameter("f32c", [n_sc, FW], f32, False)
    if USE_FP8:
        w8_ext = nc.declare_dram_parameter("w8", [128, NKC // 2, 2, NHC, 128], fp8, False)
    if USE_FP8 and VMM_FP8:
        v8_ext = nc.declare_dram_parameter(
            "v8", [128, NHC // 2, n_sc, 2, 16], fp8, False
        )
    out_ext = nc.declare_dram_parameter("out", [n_b, n_sc, 512], f32, True)

    with ExitStack() as ctx:
        tc = ctx.enter_context(tile.TileContext(nc))
        const = ctx.enter_context(tc.tile_pool(name="const", bufs=1))
        encp = ctx.enter_context(tc.tile_pool(name="enc", bufs=8))
        xp = ctx.enter_context(tc.tile_pool(name="x", bufs=8))
        bp = ctx.enter_context(tc.tile_pool(name="bt", bufs=8))
        psmm = ctx.enter_context(tc.tile_pool(name="psmm", bufs=5, space="PSUM"))
        pssc = ctx.enter_context(tc.tile_pool(name="pssc", bufs=2, space="PSUM"))
        psb = ctx.enter_context(tc.tile_pool(name="psb", bufs=1, space="PSUM"))

        cst = const.tile([128, CW], bf16, tag="cst")
        nc.sync.dma_start(cst[:], cst_ext[:])
        if USE_FP8:
            w8_sb = const.tile([128, NKC // 2, 2, NHC, 128], fp8, tag="w8")
            nc.sync.dma_start(w8_sb[:], w8_ext[:])
        if USE_FP8 and VMM_FP8:
            v8_sb = const.tile([128, NHC // 2, n_sc, 2, 16], fp8, tag="v8")
            nc.sync.dma_start(v8_sb[:], v8_ext[:])
        f32_0 = const.tile([n_sc, FW], f32, tag="f32_0")
        nc.sync.dma_start(f32_0[:], f32_ext[:])
        # re-home the f32 constants onto ScalarE: the VectorE reduce then waits
        # on the ScalarE semaphore alone (which also covers exp outputs)
        f32c = const.tile([n_sc, FW], f32, tag="f32c")
        nc.scalar.copy(f32c[:], f32_0[:])

        def cs(name, lo, width):
            return cst[:, lay[name] + lo : lay[name] + lo + width]

        # Preload the exp_and_others ACT table set (tanh+exp share it) so the
        # implicit table-load pseudo doesn't ride on a hot-loop instruction.
        warm = const.tile([1, 3], f32, tag="warm")
        nc.scalar.activation(warm[:, 0:1], f32_0[0:1, 0:1], Tanh)
        nc.scalar.activation(warm[:, 1:2], f32_0[0:1, 0:1], Exp)

        # dec projection: bias[h, b] = sum_e W_s[h,e] dec[b,e] + (b_h+b_s)[h]
        psdec = psmm.tile([128, NHC * n_b], f32, tag="ps")
        for hc in range(NHC):
            for ec in range(NEC):
                nc.tensor.matmul(
                    psdec[:, hc * n_b : (hc + 1) * n_b],
                    cs("wst", ec * H + hc * 128, 128),
                    cs("dect", ec * n_b, n_b),
                    start=(ec == 0),
                    stop=False,
                )
            nc.tensor.matmul(
                psdec[:, hc * n_b : (hc + 1) * n_b],
                cst[0:1, lay["whb"] + hc * 128 : lay["whb"] + (hc + 1) * 128],
                cst[0:1, lay["ones"] : lay["ones"] + n_b],
                start=False,
                stop=True,
            )
        # evacuate on VectorE, then touch once on ScalarE: the hot-loop tanh
        # then needs only its PE wait (hw instructions hold a single wait)
        bias_sb = const.tile([128, NHC * n_b], f32, tag="bias")
        nc.vector.tensor_copy(bias_sb[:], psdec[:])
        nc.scalar.copy(warm[:, 2:3], bias_sb[0:1, 0:1])

        vmm_dr = USE_FP8 and VMM_FP8
        GQ = 4 if n_sc % 4 == 0 else 2  # sc-group size for W-stationary reuse
        xt_dt = fp8 if vmm_dr else bf16
        for b in range(n_b):
            ps_sc = pssc.tile([16 if vmm_dr else n_sc, 512], f32, tag="ps_sc")
            for scg in range(n_sc // GQ):
                ets = []
                xts = []
                for q in range(GQ):
                    sc = scg * GQ + q
                    c0 = (b * n_sc + sc) * 512
                    et = encp.tile([128, NKC, 512], enc_dt, tag="et", name=f"et_{b}_{sc}")
                    nc.sync.dma_start(et[:], enc_ext[:, :, c0 : c0 + 512])
                    ets.append(et)
                    xts.append(
                        xp.tile([128, NHC, 512], xt_dt, tag="xt", name=f"xt_{b}_{sc}")
                    )
                for hc in range(NHC):
                    pss = [
                        psmm.tile([128, 512], f32, tag="ps", name=f"ps_{b}_{scg}_{hc}_{q}")
                        for q in range(GQ)
                    ]
                    if USE_FP8:
                        for kc2 in range(NKC // 2):
                            for q in range(GQ):
                                nc.tensor.matmul(
                                    pss[q][:],
                                    w8_sb[:, kc2, :, hc, :],
                                    ets[q][:, 2 * kc2 : 2 * kc2 + 2, :],
                                    start=(kc2 == 0),
                                    stop=(kc2 == NKC // 2 - 1),
                                    perf_mode=DR,
                                )
                    else:
                        for kc in range(NKC):
                            for q in range(GQ):
                                nc.tensor.matmul(
                                    pss[q][:],
                                    cs("wt", kc * H + hc * 128, 128),
                                    ets[q][:, kc, :],
                                    start=(kc == 0),
                                    stop=(kc == NKC - 1),
                                )
                    for q in range(GQ):
                        nc.scalar.activation(
                            xts[q][:, hc, :],
                            pss[q][:],
                            Tanh,
                            bias=bias_sb[:, hc * n_b + b : hc * n_b + b + 1],
                        )
                # v-reduction for this sc group
                for q in range(GQ):
                    sc = scg * GQ + q
                    if vmm_dr:
                        for j in range(NHC // 2):
                            nc.tensor.matmul(
                                ps_sc[:],
                                v8_sb[:, j, sc, :, :],
                                xts[q][:, 2 * j : 2 * j + 2, :],
                                start=(sc == 0 and j == 0),
                                stop=(sc == n_sc - 1 and j == NHC // 2 - 1),
                                perf_mode=DR,
                            )
                    else:
                        for hc in range(NHC):
                            nc.tensor.matmul(
                                ps_sc[:],
                                cs("vemb", (hc * n_sc + sc) * n_sc, n_sc),
                                xts[q][:, hc, :],
                                start=(sc == 0 and hc == 0),
                                stop=(sc == n_sc - 1 and hc == NHC - 1),
                            )
                # ScalarE observes its own newest tick so recycled xt slots
                # never add a second (same-engine WAW) wait to a later tanh
                nc.scalar.copy(warm[:, 2:3], xts[0][0:1, 0:1, 0:1])
            # batch epilogue: masked softmax over the n_sc*512 scores of batch b
            u8 = bp.tile([n_sc, 512], f32, tag="u8")
            nc.scalar.activation(u8[:], ps_sc[: n_sc, :], Exp)
            uw = bp.tile([n_sc, 512], f32, tag="uw")
            part = bp.tile([n_sc, 1], f32, tag="part")
            nc.vector.tensor_mul(uw[:], u8[:], f32c[:, b * 512 : (b + 1) * 512])
            nc.vector.tensor_reduce(
                part[:], uw[:], axis=mybir.AxisListType.X, op=add
            )
            tot_ps = psb.tile([n_sc, 1], f32, tag="tot")
            nc.tensor.matmul(tot_ps[:], f32c[:, n_b * 512 : n_b * 512 + n_sc], part[:])
            recip = bp.tile([n_sc, 1], f32, tag="recip")
            nc.vector.reciprocal(recip[:], tot_ps[:])
            probs = bp.tile([n_sc, 512], f32, tag="probs")
            nc.vector.tensor_scalar_mul(probs[:], uw[:], recip[:])
            nc.sync.dma_start(out_ext[b], probs[:])

    return nc


def dedupe_ldweights(nc):
    """Drop LDWEIGHTS that reload the exact stationary already resident (the
    tile legalizer emits one per matmul unconditionally).  Sync info on a
    dropped LDW is preserved on a NOP."""
    import concourse.mybir as mybir

    for fn in nc.m.functions:
        for blk in fn.blocks:
            out = []
            last_key = None
            for inst in blk.instructions:
                if isinstance(inst, mybir.InstLdweights):
                    key = (
                        str(inst.ins),
                        str(getattr(inst, "perf_mode", None)),
                        str(getattr(inst, "tile_position", None)),
                        str(getattr(inst, "tile_size", None)),
                        str(getattr(inst, "is_transpose", None)),
                    )
                    if key == last_key:
                        si = getattr(inst, "sync_info", None)
                        if si and (si.on_wait or si.on_update):
                            nop = mybir.InstNoOp(
                                name=inst.name + "-dd",
                                engine=inst.engine,
                                ins=[],
                                outs=[],
                            )
                            nop.sync_info = si
                            out.append(nop)
                        continue
                    last_key = key
                out.append(inst)
            blk.instructions = out


def legalize_single_wait(nc):
    """The walrus in this container accepts at most ONE sync wait per
    instruction and cannot encode EVENT_SEMAPHORE_RANGE_CLEAR.  Split excess
    waits onto single-wait NOPs, and replace the tile-exit range clear with
    per-semaphore decrements of the statically known final values."""
    import concourse.mybir as mybir
    import bass_rust

    m = nc.m
    totals = {}
    names = {}
    for fn in m.functions:
        for blk in fn.blocks:
            for inst in blk.instructions:
                si = getattr(inst, "sync_info", None)
                if not si:
                    continue
                for u in si.on_update or []:
                    if u.sync_type != "semaphore":
                        continue
                    v = u.update_value if u.update_value is not None else 1
                    if u.update_mode in ("sem-inc", "sem-add-imm"):
                        totals[u.id] = totals.get(u.id, 0) + v
                    elif u.update_mode in ("sem-dec", "sem-sub-imm"):
                        totals[u.id] = totals.get(u.id, 0) - v
                    names[u.id] = u.ant_name

    nid = [0]

    def mk_nop(engine, wait):
        nid[0] += 1
        nop = mybir.InstNoOp(name=f"I-lsw-{nid[0]}", engine=engine, ins=[], outs=[])
        nop.sync_info = bass_rust.SyncInfo(on_wait=[wait], on_update=[])
        return nop

    def mk_dec(engine, sem_id, value):
        nid[0] += 1
        es = mybir.InstEventSemaphore(
            name=f"I-lsc-{nid[0]}", engine=engine, ins=[], outs=[]
        )
        u = bass_rust.SyncUpdate(
            sync_type="semaphore",
            id=sem_id,
            ant_name=names.get(sem_id, f"sem{sem_id}"),
            update_mode="sem-sub-imm",
            update_value=value,
            update_reg=None,
        )
        es.sync_info = bass_rust.SyncInfo(on_wait=[], on_update=[u])
        return es

    for fn in m.functions:
        for blk in fn.blocks:
            out = []
            for inst in blk.instructions:
                if (
                    isinstance(inst, mybir.InstISA)
                    and getattr(inst, "isa_opcode", None) == 176
                ):
                    first = getattr(inst, "range_first", None)
                    last = getattr(inst, "range_last", None)
                    if first is None:
                        d = inst.concise()
                        import re

                        first = int(re.search(r"range_first=(\d+)", d).group(1))
                        last = int(re.search(r"range_last=(\d+)", d).group(1))
                    for sem_id in range(first, last + 1):
                        v = totals.get(sem_id, 0)
                        if v > 0:
                            out.append(mk_dec(inst.engine, sem_id, v))
                    continue
                si = getattr(inst, "sync_info", None)
                waits = list(si.on_wait) if si and si.on_wait else []
                if len(waits) > 1:
                    for w in waits[:-1]:
                        out.append(mk_nop(inst.engine, w))
                    inst.sync_info = bass_rust.SyncInfo(
                        on_wait=[waits[-1]], on_update=list(si.on_update or [])
                    )
                out.append(inst)
            blk.instructions = out


def prep_w8(W_h_w):
    """DoubleRow fp8 weights: w8[p, kc2, i, hc, h] = W_h[hc*128+h, kc2*256+i*128+p]."""
    Wh = np.asarray(W_h_w, np.float32)  # [H, E]
    return np.ascontiguousarray(
        Wh.T.reshape(NKC // 2, 2, 128, NHC, 128).transpose(2, 0, 1, 3, 4)
    ).astype(FP8)


def prep_v8(v_w, n_sc=NSC):
    """DoubleRow fp8 v embedding: v8[p, j, sc, i, m] = (m==sc) v[(2j+i)*128+p]."""
    v = np.asarray(v_w, np.float32).reshape(H)
    v8 = np.zeros((128, NHC // 2, n_sc, 2, 16), np.float32)
    vr = v.reshape(NHC // 2, 2, 128)  # [j, i, p]
    for sc in range(n_sc):
        v8[:, :, sc, :, sc] = vr.transpose(2, 0, 1)
    return v8.astype(FP8)


def prep_shared(W_h_w, W_h_b, W_s_w, W_s_b, v_w, n_b=NB, n_sc=NSC):
    """The shared part of the constants tensor (zero where dect goes)."""
    lay = const_layout(n_b, n_sc)
    Wh = np.asarray(W_h_w, np.float32)  # [H, E]
    Ws = np.asarray(W_s_w, np.float32)  # [H, H]
    v = np.asarray(v_w, np.float32).reshape(H)

    cst = np.zeros((128, lay["_width"]), np.float32)
    # wt[p, kc*H + h] = Wh[h, kc*128+p]
    if not USE_FP8:
        cst[:, lay["wt"] : lay["wt"] + NKC * H] = (
            Wh.T.reshape(NKC, 128, H).transpose(1, 0, 2).reshape(128, NKC * H)
        )
    # vemb[p, (hc*n_sc+sc)*n_sc + m] = (m==sc) * v[hc*128+p]
    vr = v.reshape(NHC, 128).T  # [p, hc]
    vemb = np.zeros((128, NHC, n_sc, n_sc), np.float32)
    for scm in range(n_sc):
        vemb[:, :, scm, scm] = vr
    cst[:, lay["vemb"] : lay["vemb"] + NHC * n_sc * n_sc] = vemb.reshape(128, -1)
    # wst[p, ec*H + hc*128 + h] = Ws[hc*128+h, ec*128+p]
    cst[:, lay["wst"] : lay["wst"] + NEC * H] = (
        Ws.T.reshape(NEC, 128, NHC, 128).transpose(1, 0, 2, 3).reshape(128, NEC * H)
    )
    # whb row 0: combined bias, ones row 0
    cst[0, lay["whb"] : lay["whb"] + H] = np.asarray(W_h_b, np.float32) + np.asarray(
        W_s_b, np.float32
    )
    cst[0, lay["ones"] : lay["ones"] + n_b] = 1.0
    return cst


def prep_core(cst_shared, dec_shard, enc_shard, mask_shard, n_b=NB, n_sc=NSC):
    """Per-core input map (layout transforms + bf16 cast only)."""
    lay = const_layout(n_b, n_sc)
    ncols = n_b * n_sc * 512
    dt = FP8 if USE_FP8 else BF16
    rows = np.asarray(enc_shard, np.float32).reshape(ncols, E).astype(dt)
    # encT[p, kc, col] = enc[col, kc*128+p]
    encT = np.ascontiguousarray(rows.T.reshape(NKC, 128, ncols).transpose(1, 0, 2))
    cst = cst_shared.copy()
    # dect[p, ec*n_b + b] = dec[b, ec*128+p]
    cst[:, lay["dect"] : lay["dect"] + NEC * n_b] = (
        np.asarray(dec_shard, np.float32)
        .T.reshape(NEC, 128, n_b)
        .transpose(1, 0, 2)
        .reshape(128, NEC * n_b)
    )
    # f32c: mask columns then ones columns
    f32c = np.ones((n_sc, n_b * 512 + n_sc), np.float32)
    f32c[:, : n_b * 512] = (
        np.asarray(mask_shard)
        .astype(np.float32)
        .reshape(n_b, n_sc, 512)
        .transpose(1, 0, 2)
        .reshape(n_sc, n_b * 512)
    )
    return {"encT": encT, "consts": cst.astype(BF16), "f32c": f32c}


def kernel(decoder_output, encoder_output, mask, W_h_w, W_h_b, W_s_w, W_s_b, v_w):
    global LAST_EXEC_NS, LAST_RESULTS
    from concourse.bass_utils import run_bass_kernel_spmd

    if "nc" not in _CACHE:
        nc0 = build_bass()
        nc0.finalize()
        dedupe_ldweights(nc0)
        legalize_single_wait(nc0)
        _CACHE["nc"] = nc0
    nc = _CACHE["nc"]

    shared = prep_shared(W_h_w, W_h_b, W_s_w, W_s_b, v_w)
    w8 = prep_w8(W_h_w) if USE_FP8 else None
    v8 = prep_v8(v_w) if (USE_FP8 and VMM_FP8) else None
    in_maps = []
    for c in range(NCORES):
        sl = slice(c * NB, (c + 1) * NB)
        m = prep_core(shared, decoder_output[sl], encoder_output[sl], mask[sl])
        if USE_FP8:
            m["w8"] = w8
        if v8 is not None:
            m["v8"] = v8
        in_maps.append(m)

    res = run_bass_kernel_spmd(nc, in_maps, core_ids=list(range(NCORES)), trace=TRACE)
    if TRACE:
        LAST_EXEC_NS = res.exec_time_ns
        LAST_RESULTS = res
    out = np.concatenate(
        [
            np.asarray(res.results[c]["out"], np.float32).reshape(NB, S)
            for c in range(NCORES)
        ],
        axis=0,
    )
    return out
